# revision 12
# baseline (speedup 1.0000x reference)
"""Trainium2 Bass kernel for nn_Block_54382875902076 (dense transformer block).

Reference computation (B=4, S=2048, E=512, H=8, D=64, fp32):
    res = x
    h   = LN1(x)                      (no bias, eps=1e-6)
    h   = res + Attn(h)               (causal, wo1 [H,D,E] then wo2 [E,E])
    h   = LN2(h)
    out = res + gelu(h @ w1) @ w2     (NOTE: res = block input, both residuals)

Sharding (8 cores): core c = (batch b = c//2, head-group g = c%2).
Each core computes LN1 + QKV for its 4 heads over the full sequence,
exact-causal attention (identical static structure on all cores), the wo1
partial projection, then pair-wise ReduceScatters (pipelined per 512-token
block, issued one block behind attention) sum the two head-groups' partials
and hand each core half of every block's rows for wo2 + LN2 + MLP.

All matmuls run in bf16 (fp32 matmul is 4x slower on the PE); LN statistics
and softmax accumulation stay fp32. LN scales are folded into wq/wk/wv (ln1)
and w1 (ln2) on the host. Weights are pre-arranged on the host into the
[128, k, m] SBUF layout so every load is one contiguous DMA.
"""

import functools
import sys

import numpy as np

for _p in ("/opt/trn_rl_repo", "/root/.axon_site/_ro/trn_rl_repo"):
    if _p not in sys.path:
        sys.path.append(_p)

import ml_dtypes  # noqa: E402
import concourse.bass as bass  # noqa: E402
import concourse.tile as tile  # noqa: E402
from concourse import bacc, mybir  # noqa: E402
from concourse.bass_utils import run_bass_kernel_spmd  # noqa: E402

F32 = mybir.dt.float32
BF16 = mybir.dt.bfloat16
AF = mybir.ActivationFunctionType
ALU = mybir.AluOpType

B, S, E, H, D = 4, 2048, 512, 8, 64
HG = H // 2            # heads per core
SQ = S // 2            # rows per core after reduce-scatter
NT = S // 128          # 16 token tiles (full seq)
NTQ = SQ // 128        # 8 token tiles (own half)
QTS = S // 512         # 4 q-tiles of 512 for attention


def _build_graph():
    nc = bacc.Bacc("TRN2", target_bir_lowering=False, debug=False, num_devices=8)

    xf = nc.declare_dram_parameter("xf", [S, E], F32, isOutput=False)
    xq = nc.declare_dram_parameter("xq", [128, NTQ, E], F32, isOutput=False)
    wq = nc.declare_dram_parameter("wq", [128, 4, HG * D], BF16, isOutput=False)
    wk = nc.declare_dram_parameter("wk", [128, 4, HG * D], BF16, isOutput=False)
    wv = nc.declare_dram_parameter("wv", [128, 4, HG * D], BF16, isOutput=False)
    wo1 = nc.declare_dram_parameter("wo1", [128, 2, E], BF16, isOutput=False)
    wo2 = nc.declare_dram_parameter("wo2", [128, 4, E], BF16, isOutput=False)
    w1 = nc.declare_dram_parameter("w1", [128, 4, 4 * E], BF16, isOutput=False)
    w2 = nc.declare_dram_parameter("w2", [128, 16, E], BF16, isOutput=False)
    masks = nc.declare_dram_parameter("masks", [128, 4, 512], BF16, isOutput=False)
    out = nc.declare_dram_parameter("out", [SQ, E], F32, isOutput=True)

    with tile.TileContext(nc) as tc:
        with (
            tc.tile_pool(name="consts", bufs=1) as consts,
            tc.tile_pool(name="acts", bufs=1) as acts,
            tc.tile_pool(name="work", bufs=3) as work,
            tc.tile_pool(name="stats", bufs=6) as stats,
            tc.tile_pool(name="expp", bufs=3) as expp,
            tc.tile_pool(name="psA", bufs=2, space="PSUM") as psA,
            tc.tile_pool(name="psB", bufs=2, space="PSUM") as psB,
            tc.tile_pool(name="psC", bufs=2, space="PSUM") as psC,
            tc.tile_pool(name="dram", bufs=1, space="DRAM") as dram,
        ):
            # ---- constants / weights (contiguous loads, gpsimd queue) ----
            eps_t = consts.tile([128, 1], F32)
            nc.vector.memset(eps_t, 1e-6)

            def load_const(shape, src, tag):
                t = consts.tile(shape, BF16, tag=tag)
                nc.gpsimd.dma_start(t[:], src[:])
                return t

            # only what the LN1/QKV/attention prologue needs; big late-phase
            # weights (wo2/w1/w2/xq) are loaded mid-attention to keep HBM
            # bandwidth free for the x tiles at kernel start
            wq_sb = load_const([128, 4, HG * D], wq, "wq_sb")
            wk_sb = load_const([128, 4, HG * D], wk, "wk_sb")
            wv_sb = load_const([128, 4, HG * D], wv, "wv_sb")
            wo1_sb = load_const([128, 2, E], wo1, "wo1_sb")
            masks_sb = load_const([128, 4, 512], masks, "masks_sb")

            def layernorm_tile(src_ap, dst_tile):
                """dst (bf16) = (src - mean) * rsqrt(var + eps); stats in fp32."""
                st6 = stats.tile([128, 6], F32, tag="st6")
                nc.vector.bn_stats(st6[:], src_ap)
                mv = stats.tile([128, 2], F32, tag="mv")
                nc.vector.bn_aggr(mv[:], st6[:])
                std = stats.tile([128, 1], F32, tag="std")
                nc.scalar.activation(std[:], mv[:, 1:2], AF.Sqrt, bias=eps_t[:])
                rsig = stats.tile([128, 1], F32, tag="rsig")
                nc.vector.reciprocal(rsig[:], std[:])
                nc.vector.tensor_scalar(
                    dst_tile[:], src_ap, mv[:, 0:1], rsig[:],
                    op0=ALU.subtract, op1=ALU.mult,
                )

            # ---- LN1 + per-block transpose + QKV -------------------------
            h1_dram = dram.tile([S, E], BF16)
            h1T = acts.tile([128, 4, S], BF16)
            KT = acts.tile([128, 2, S], BF16)
            QT = acts.tile([128, 2, S], BF16)
            V65 = acts.tile([128, NT, HG, D + 1], BF16)
            nc.vector.memset(V65[:, :, :, D:D + 1], 1.0)
            for st in range(4):
                for t in range(4 * st, 4 * st + 4):
                    xt = work.tile([128, E], F32, tag="wf32")
                    nc.sync.dma_start(xt[:], xf[t * 128:(t + 1) * 128, :])
                    h1t = work.tile([128, E], BF16, tag="wbf")
                    layernorm_tile(xt[:], h1t)
                    nc.sync.dma_start(h1_dram[t * 128:(t + 1) * 128, :], h1t[:])
                sl = slice(st * 512, (st + 1) * 512)
                nc.sync.dma_start_transpose(h1T[:, :, sl], h1_dram[sl, :])
                for mi in range(2):
                    for dst, w_sb in ((KT, wk_sb), (QT, wq_sb)):
                        ps = psC.tile([128, 512], F32, tag="psC")
                        for ko in range(4):
                            nc.tensor.matmul(
                                ps[:],
                                lhsT=w_sb[:, ko, mi * 128:(mi + 1) * 128],
                                rhs=h1T[:, ko, sl],
                                start=(ko == 0), stop=(ko == 3),
                            )
                        nc.vector.tensor_copy(dst[:, mi, sl], ps[:])
                for tt in range(4 * st, 4 * st + 4):
                    ps = psC.tile([128, 512], F32, tag="psC")
                    for ko in range(4):
                        nc.tensor.matmul(
                            ps[:, 0:HG * D],
                            lhsT=h1T[:, ko, tt * 128:(tt + 1) * 128],
                            rhs=wv_sb[:, ko, :],
                            start=(ko == 0), stop=(ko == 3),
                        )
                    nc.vector.tensor_copy(
                        V65[:, tt, :, 0:D],
                        ps[:, 0:HG * D].rearrange("p (h d) -> p h d", h=HG),
                    )

            # ---- causal attention + one-behind wo1/ReduceScatter ---------
            # attnT is per-qt so wo1(qt-1) never waits on attention(qt) writes
            o1_dram = dram.tile([S, E], BF16)
            o1r_dram = dram.tile([SQ, E], BF16)
            attnT_tiles = {}
            o1rT_tiles = {}

            def attention_block(qt):
                attnT = acts.tile([128, 2, 512], BF16, tag=f"attnT{qt}")
                attnT_tiles[qt] = attnT
                ext = 4 * (qt + 1)           # causal extent in 128-chunks
                for a in range(2):           # local head pairs (2a, 2a+1)
                    avA = psB.tile([D + 1, 512], F32, tag="psB")
                    avB = psB.tile([D + 1, 512], F32, tag="psB")
                    exs = [None] * ext
                    # software-pipelined: AV of chunk c-1 issues after the
                    # scores of chunk c, so the PE never stalls on the exp
                    for c in range(ext + 1):
                        if c < ext:
                            sp = psA.tile([128, 1024], F32, tag="psA")
                            nc.tensor.matmul(
                                sp[:, 0:512],
                                lhsT=KT[0:64, a, c * 128:(c + 1) * 128],
                                rhs=QT[0:64, a, qt * 512:(qt + 1) * 512],
                                start=True, stop=True,
                            )
                            nc.tensor.matmul(
                                sp[:, 512:1024],
                                lhsT=KT[64:128, a, c * 128:(c + 1) * 128],
                                rhs=QT[64:128, a, qt * 512:(qt + 1) * 512],
                                start=True, stop=True,
                            )
                            ex = expp.tile([128, 1024], BF16, tag="ex")
                            nc.scalar.activation(ex[:], sp[:], AF.Exp, scale=D ** -0.5)
                            j = c - 4 * qt
                            if j >= 0:       # diagonal chunk: apply causal mask
                                nc.vector.tensor_mul(ex[:, 0:512], ex[:, 0:512], masks_sb[:, j, :])
                                nc.vector.tensor_mul(ex[:, 512:1024], ex[:, 512:1024], masks_sb[:, j, :])
                            exs[c] = ex
                        if c >= 1:
                            ex = exs[c - 1]
                            nc.tensor.matmul(
                                avA[:], lhsT=V65[:, c - 1, 2 * a, :], rhs=ex[:, 0:512],
                                start=(c == 1), stop=(c == ext),
                            )
                            nc.tensor.matmul(
                                avB[:], lhsT=V65[:, c - 1, 2 * a + 1, :], rhs=ex[:, 512:1024],
                                start=(c == 1), stop=(c == ext),
                            )
                    for hh, av in ((2 * a, avA), (2 * a + 1, avB)):
                        # copy PSUM out quickly, then build 1/denominator with
                        # the free dim spread across partitions (reciprocal is
                        # ~6.5 cyc per free-elem per lane, so [1,512] is slow)
                        avs = work.tile([D + 1, 512], F32, tag="avs")
                        nc.vector.tensor_copy(avs[:], av[:])
                        d4 = stats.tile([128, 4], F32, tag="d4")
                        nc.sync.dma_start(
                            d4[:], avs[D:D + 1, :].rearrange("o (p f) -> o p f", p=128)
                        )
                        r4 = stats.tile([128, 4], F32, tag="r4")
                        nc.vector.reciprocal(r4[:], d4[:])
                        rrow = stats.tile([1, 512], F32, tag="rrow")
                        nc.sync.dma_start(
                            rrow.rearrange("o (p f) -> o p f", p=128), r4[:]
                        )
                        den_b = work.tile([64, 512], F32, tag="denb")
                        nc.gpsimd.partition_broadcast(den_b[:], rrow[0:1, :], channels=64)
                        if hh % 2 == 0:
                            nc.vector.tensor_tensor(
                                attnT[0:64, a, :], avs[0:D, :], den_b[:], op=ALU.mult,
                            )
                        else:
                            tmp = work.tile([64, 512], BF16, tag="atmp")
                            nc.vector.tensor_tensor(tmp[:], avs[0:D, :], den_b[:], op=ALU.mult)
                            nc.sync.dma_start(attnT[64:128, a, :], tmp[:])

            def wo1_rs_block(qt):
                attnT = attnT_tiles[qt]
                for lt in range(4):
                    tt = 4 * qt + lt
                    ps = psC.tile([128, 512], F32, tag="psC")
                    for ko in range(2):
                        nc.tensor.matmul(
                            ps[:],
                            lhsT=attnT[:, ko, lt * 128:(lt + 1) * 128],
                            rhs=wo1_sb[:, ko, :],
                            start=(ko == 0), stop=(ko == 1),
                        )
                    o1t = work.tile([128, E], BF16, tag="wbf")
                    nc.vector.tensor_copy(o1t[:], ps[:])
                    nc.sync.dma_start(o1_dram[tt * 128:(tt + 1) * 128, :], o1t[:])
                nc.gpsimd.collective_compute(
                    "ReduceScatter", ALU.add,
                    replica_groups=[[0, 1], [2, 3], [4, 5], [6, 7]],
                    ins=[o1_dram[qt * 512:(qt + 1) * 512, :].opt()],
                    outs=[o1r_dram[qt * 256:(qt + 1) * 256, :].opt()],
                )
                o1rT = acts.tile([128, 4, 256], BF16, tag=f"o1rT{qt}")
                o1rT_tiles[qt] = o1rT
                nc.sync.dma_start_transpose(
                    o1rT[:], o1r_dram[qt * 256:(qt + 1) * 256, :],
                )

            for qt in range(QTS):
                attention_block(qt)
                if qt == 0:
                    # late-phase weights: load now so the DMAs don't compete
                    # with the x/LN1 stream at kernel start
                    wo2_sb = load_const([128, 4, E], wo2, "wo2_sb")
                    w1_sb = load_const([128, 4, 4 * E], w1, "w1_sb")
                    w2_sb = load_const([128, 16, E], w2, "w2_sb")
                    xq_sb = acts.tile([128, NTQ, E], F32)
                    nc.gpsimd.dma_start(xq_sb[:], xq[:])
                if qt >= 1:
                    wo1_rs_block(qt - 1)   # one block behind: keeps PE stream unblocked
            wo1_rs_block(QTS - 1)

            # ---- wo2 + residual + LN2 (all blocks), then MLP -------------
            # h2T/m1T are per-half tiles so first-half MLP matmuls never wait
            # on second-half LayerNorm chains
            h2_dram = dram.tile([SQ, E], BF16)
            h2T_tiles = [
                acts.tile([128, 4, 512], BF16, tag=f"h2T{h}", name=f"h2T{h}")
                for h in range(2)
            ]
            m1T_tiles = [
                acts.tile([128, 16, 512], BF16, tag=f"m1T{h}", name=f"m1T{h}")
                for h in range(2)
            ]
            for tt in range(NTQ):
                ps = psC.tile([128, 512], F32, tag="psC")
                o1rT = o1rT_tiles[tt // 2]
                for ko in range(4):
                    nc.tensor.matmul(
                        ps[:],
                        lhsT=o1rT[:, ko, (tt % 2) * 128:(tt % 2 + 1) * 128],
                        rhs=wo2_sb[:, ko, :],
                        start=(ko == 0), stop=(ko == 3),
                    )
                h2r = work.tile([128, E], F32, tag="wf32")
                nc.vector.tensor_add(h2r[:], ps[:], xq_sb[:, tt, :])
                h2t = work.tile([128, E], BF16, tag="wbf")
                layernorm_tile(h2r[:], h2t)
                nc.sync.dma_start(h2_dram[tt * 128:(tt + 1) * 128, :], h2t[:])
                if tt % 4 == 3:
                    half = tt // 4
                    hsl = slice(half * 512, (half + 1) * 512)
                    nc.sync.dma_start_transpose(h2T_tiles[half][:], h2_dram[hsl, :])
            for half in range(2):
                h2T = h2T_tiles[half]
                m1T = m1T_tiles[half]
                for mi in range(16):
                    ps = psC.tile([128, 512], F32, tag="psC")
                    for ko in range(4):
                        nc.tensor.matmul(
                            ps[:],
                            lhsT=w1_sb[:, ko, mi * 128:(mi + 1) * 128],
                            rhs=h2T[:, ko, :],
                            start=(ko == 0), stop=(ko == 3),
                        )
                    nc.scalar.activation(m1T[:, mi, :], ps[:], AF.Gelu_apprx_tanh)
                for lt in range(4):
                    tt = 4 * half + lt
                    ps = psC.tile([128, 512], F32, tag="psC")
                    for ko in range(16):
                        nc.tensor.matmul(
                            ps[:],
                            lhsT=m1T[:, ko, lt * 128:(lt + 1) * 128],
                            rhs=w2_sb[:, ko, :],
                            start=(ko == 0), stop=(ko == 15),
                        )
                    ot = work.tile([128, E], F32, tag="wf32")
                    nc.vector.tensor_add(ot[:], ps[:], xq_sb[:, tt, :])
                    nc.sync.dma_start(out[tt * 128:(tt + 1) * 128, :], ot[:])

    nc.finalize()
    return nc


@functools.lru_cache(maxsize=1)
def _get_graph():
    return _build_graph()


def _bf16_kpm(a, p=128):
    """[K, M] fp32 -> contiguous [p, K//p, M] bf16 (SBUF (k p) layout)."""
    k, m = a.shape
    return np.ascontiguousarray(
        a.reshape(k // p, p, m).transpose(1, 0, 2)
    ).astype(ml_dtypes.bfloat16)


def _own_rows(rank):
    """Global row indices owned by a core after the per-block reduce-scatters."""
    return np.concatenate(
        [np.arange(512 * qt + 256 * rank, 512 * qt + 256 * rank + 256) for qt in range(QTS)]
    )


def _make_in_maps(x, wq, wk, wv, wo1, wo2, w1, w2, ln1_scale, ln2_scale):
    x = np.asarray(x, dtype=np.float32)
    wq = np.asarray(wq, dtype=np.float32).reshape(E, H * D)
    wk = np.asarray(wk, dtype=np.float32).reshape(E, H * D)
    wv = np.asarray(wv, dtype=np.float32).reshape(E, H * D)
    wo1 = np.asarray(wo1, dtype=np.float32).reshape(H * D, E)
    wo2 = np.asarray(wo2, dtype=np.float32)
    w1 = np.asarray(w1, dtype=np.float32)
    w2 = np.asarray(w2, dtype=np.float32)
    s1 = np.asarray(ln1_scale, dtype=np.float32)[:, None]
    s2 = np.asarray(ln2_scale, dtype=np.float32)[:, None]

    wq_s, wk_s, wv_s = s1 * wq, s1 * wk, s1 * wv
    w1_s = s2 * w1

    # causal mask patterns for diagonal 128-chunks within a 512 q-tile:
    # mask_j[p, f] = 1.0 iff (128j + p) <= f;  stored [p, j, f]
    iota_p = np.arange(128)[:, None]
    iota_f = np.arange(512)[None, :]
    mask_np = np.ascontiguousarray(np.stack(
        [(128 * j + iota_p <= iota_f).astype(np.float32) for j in range(4)]
    ).transpose(1, 0, 2)).astype(ml_dtypes.bfloat16)

    in_maps = []
    for c in range(8):
        b, g = c // 2, c % 2
        hd = slice(g * HG * D, (g + 1) * HG * D)
        rows = _own_rows(c % 2)
        xq_arr = np.ascontiguousarray(
            x[b][rows].reshape(NTQ, 128, E).transpose(1, 0, 2)
        )
        in_maps.append({
            "xf": np.ascontiguousarray(x[b]),
            "xq": xq_arr,
            "wq": _bf16_kpm(wq_s[:, hd]),
            "wk": _bf16_kpm(wk_s[:, hd]),
            "wv": _bf16_kpm(wv_s[:, hd]),
            "wo1": _bf16_kpm(wo1[hd, :]),
            "wo2": _bf16_kpm(wo2),
            "w1": _bf16_kpm(w1_s),
            "w2": _bf16_kpm(w2),
            "masks": mask_np,
        })
    return in_maps


def run(trace=False, **inputs):
    nc = _get_graph()
    in_maps = _make_in_maps(**inputs)
    res = run_bass_kernel_spmd(nc, in_maps, core_ids=list(range(8)), trace=trace)
    y = np.empty((B, S, E), dtype=np.float32)
    for c in range(8):
        b = c // 2
        y[b][_own_rows(c % 2)] = res.results[c]["out"]
    return y, res


def kernel(**inputs):
    y, _ = run(trace=False, **inputs)
    return y


# revision 13
# speedup vs baseline: 1.0159x; 1.0159x over previous
"""Trainium2 Bass kernel for nn_Block_54382875902076 (dense transformer block).

Reference computation (B=4, S=2048, E=512, H=8, D=64, fp32):
    res = x
    h   = LN1(x)                      (no bias, eps=1e-6)
    h   = res + Attn(h)               (causal, wo1 [H,D,E] then wo2 [E,E])
    h   = LN2(h)
    out = res + gelu(h @ w1) @ w2     (NOTE: res = block input, both residuals)

Sharding (8 cores): core c = (batch b = c//2, head-group g = c%2).
Each core computes LN1 + QKV for its 4 heads over the full sequence,
exact-causal attention (identical static structure on all cores), the wo1
partial projection, then pair-wise ReduceScatters (pipelined per 512-token
block, issued one block behind attention) sum the two head-groups' partials
and hand each core half of every block's rows for wo2 + LN2 + MLP.

All matmuls run in bf16 (fp32 matmul is 4x slower on the PE); LN statistics
and softmax accumulation stay fp32. LN scales are folded into wq/wk/wv (ln1)
and w1 (ln2) on the host. Weights are pre-arranged on the host into the
[128, k, m] SBUF layout so every load is one contiguous DMA.
"""

import functools
import sys

import numpy as np

for _p in ("/opt/trn_rl_repo", "/root/.axon_site/_ro/trn_rl_repo"):
    if _p not in sys.path:
        sys.path.append(_p)

import ml_dtypes  # noqa: E402
import concourse.bass as bass  # noqa: E402
import concourse.tile as tile  # noqa: E402
from concourse import bacc, mybir  # noqa: E402
from concourse.bass_utils import run_bass_kernel_spmd  # noqa: E402

F32 = mybir.dt.float32
BF16 = mybir.dt.bfloat16
AF = mybir.ActivationFunctionType
ALU = mybir.AluOpType

B, S, E, H, D = 4, 2048, 512, 8, 64
HG = H // 2            # heads per core
SQ = S // 2            # rows per core after reduce-scatter
NT = S // 128          # 16 token tiles (full seq)
NTQ = SQ // 128        # 8 token tiles (own half)
QTS = S // 512         # 4 q-tiles of 512 for attention


def _build_graph():
    nc = bacc.Bacc("TRN2", target_bir_lowering=False, debug=False, num_devices=8)

    xf = nc.declare_dram_parameter("xf", [S, E], F32, isOutput=False)
    xq = nc.declare_dram_parameter("xq", [128, NTQ, E], F32, isOutput=False)
    wq = nc.declare_dram_parameter("wq", [128, 4, HG * D], BF16, isOutput=False)
    wk = nc.declare_dram_parameter("wk", [128, 4, HG * D], BF16, isOutput=False)
    wv = nc.declare_dram_parameter("wv", [128, 4, HG * D], BF16, isOutput=False)
    wo1 = nc.declare_dram_parameter("wo1", [128, 2, E], BF16, isOutput=False)
    wo2 = nc.declare_dram_parameter("wo2", [128, 4, E], BF16, isOutput=False)
    w1 = nc.declare_dram_parameter("w1", [128, 4, 4 * E], BF16, isOutput=False)
    w2 = nc.declare_dram_parameter("w2", [128, 16, E], BF16, isOutput=False)
    masks = nc.declare_dram_parameter("masks", [128, 4, 512], BF16, isOutput=False)
    out = nc.declare_dram_parameter("out", [SQ, E], F32, isOutput=True)

    with tile.TileContext(nc) as tc:
        with (
            tc.tile_pool(name="consts", bufs=1) as consts,
            tc.tile_pool(name="acts", bufs=1) as acts,
            tc.tile_pool(name="work", bufs=3) as work,
            tc.tile_pool(name="stats", bufs=6) as stats,
            tc.tile_pool(name="expp", bufs=3) as expp,
            tc.tile_pool(name="psA", bufs=2, space="PSUM") as psA,
            tc.tile_pool(name="psB", bufs=2, space="PSUM") as psB,
            tc.tile_pool(name="psC", bufs=2, space="PSUM") as psC,
            tc.tile_pool(name="dram", bufs=1, space="DRAM") as dram,
        ):
            # ---- constants / weights (contiguous loads, gpsimd queue) ----
            eps_t = consts.tile([128, 1], F32)
            nc.vector.memset(eps_t, 1e-6)

            def load_const(shape, src, tag):
                # scalar-engine HWDGE queue: fast path, and keeps the sync
                # queue free for the x/h1 stream at kernel start
                t = consts.tile(shape, BF16, tag=tag)
                nc.scalar.dma_start(t[:], src[:])
                return t

            # only what the LN1/QKV/attention prologue needs; big late-phase
            # weights (wo2/w1/w2/xq) are loaded mid-attention to keep HBM
            # bandwidth free for the x tiles at kernel start
            wq_sb = load_const([128, 4, HG * D], wq, "wq_sb")
            wk_sb = load_const([128, 4, HG * D], wk, "wk_sb")
            wv_sb = load_const([128, 4, HG * D], wv, "wv_sb")
            wo1_sb = load_const([128, 2, E], wo1, "wo1_sb")
            masks_sb = load_const([128, 4, 512], masks, "masks_sb")

            def layernorm_tile(src_ap, dst_tile):
                """dst (bf16) = (src - mean) * rsqrt(var + eps); stats in fp32."""
                st6 = stats.tile([128, 6], F32, tag="st6")
                nc.vector.bn_stats(st6[:], src_ap)
                mv = stats.tile([128, 2], F32, tag="mv")
                nc.vector.bn_aggr(mv[:], st6[:])
                std = stats.tile([128, 1], F32, tag="std")
                nc.scalar.activation(std[:], mv[:, 1:2], AF.Sqrt, bias=eps_t[:])
                rsig = stats.tile([128, 1], F32, tag="rsig")
                nc.vector.reciprocal(rsig[:], std[:])
                nc.vector.tensor_scalar(
                    dst_tile[:], src_ap, mv[:, 0:1], rsig[:],
                    op0=ALU.subtract, op1=ALU.mult,
                )

            # ---- LN1 + per-block transpose + QKV -------------------------
            h1_dram = dram.tile([S, E], BF16)
            h1T = acts.tile([128, 4, S], BF16)
            KT = acts.tile([128, 2, S], BF16)
            QT = acts.tile([128, 2, S], BF16)
            V65 = acts.tile([128, NT, HG, D + 1], BF16)
            nc.vector.memset(V65[:, :, :, D:D + 1], 1.0)
            for st in range(4):
                for t in range(4 * st, 4 * st + 4):
                    xt = work.tile([128, E], F32, tag="wf32")
                    nc.sync.dma_start(xt[:], xf[t * 128:(t + 1) * 128, :])
                    h1t = work.tile([128, E], BF16, tag="wbf")
                    layernorm_tile(xt[:], h1t)
                    nc.sync.dma_start(h1_dram[t * 128:(t + 1) * 128, :], h1t[:])
                sl = slice(st * 512, (st + 1) * 512)
                nc.sync.dma_start_transpose(h1T[:, :, sl], h1_dram[sl, :])
                for mi in range(2):
                    for dst, w_sb in ((KT, wk_sb), (QT, wq_sb)):
                        ps = psC.tile([128, 512], F32, tag="psC")
                        for ko in range(4):
                            nc.tensor.matmul(
                                ps[:],
                                lhsT=w_sb[:, ko, mi * 128:(mi + 1) * 128],
                                rhs=h1T[:, ko, sl],
                                start=(ko == 0), stop=(ko == 3),
                            )
                        nc.vector.tensor_copy(dst[:, mi, sl], ps[:])
                for tt in range(4 * st, 4 * st + 4):
                    ps = psC.tile([128, 512], F32, tag="psC")
                    for ko in range(4):
                        nc.tensor.matmul(
                            ps[:, 0:HG * D],
                            lhsT=h1T[:, ko, tt * 128:(tt + 1) * 128],
                            rhs=wv_sb[:, ko, :],
                            start=(ko == 0), stop=(ko == 3),
                        )
                    nc.vector.tensor_copy(
                        V65[:, tt, :, 0:D],
                        ps[:, 0:HG * D].rearrange("p (h d) -> p h d", h=HG),
                    )

            # ---- causal attention + one-behind wo1/ReduceScatter ---------
            # attnT is per-qt so wo1(qt-1) never waits on attention(qt) writes
            o1_dram = dram.tile([S, E], BF16)
            o1r_dram = dram.tile([SQ, E], BF16)
            attnT_tiles = {}
            o1rT_tiles = {}

            def attention_block(qt):
                attnT = acts.tile([128, 2, 512], BF16, tag=f"attnT{qt}")
                attnT_tiles[qt] = attnT
                ext = 4 * (qt + 1)           # causal extent in 128-chunks
                for a in range(2):           # local head pairs (2a, 2a+1)
                    avA = psB.tile([D + 1, 512], F32, tag="psB")
                    avB = psB.tile([D + 1, 512], F32, tag="psB")
                    exs = [None] * ext
                    # software-pipelined: AV of chunk c-1 issues after the
                    # scores of chunk c, so the PE never stalls on the exp
                    for c in range(ext + 1):
                        if c < ext:
                            sp = psA.tile([128, 1024], F32, tag="psA")
                            nc.tensor.matmul(
                                sp[:, 0:512],
                                lhsT=KT[0:64, a, c * 128:(c + 1) * 128],
                                rhs=QT[0:64, a, qt * 512:(qt + 1) * 512],
                                start=True, stop=True,
                            )
                            nc.tensor.matmul(
                                sp[:, 512:1024],
                                lhsT=KT[64:128, a, c * 128:(c + 1) * 128],
                                rhs=QT[64:128, a, qt * 512:(qt + 1) * 512],
                                start=True, stop=True,
                            )
                            ex = expp.tile([128, 1024], BF16, tag="ex")
                            nc.scalar.activation(ex[:], sp[:], AF.Exp, scale=D ** -0.5)
                            j = c - 4 * qt
                            if j >= 0:       # diagonal chunk: apply causal mask
                                nc.vector.tensor_mul(ex[:, 0:512], ex[:, 0:512], masks_sb[:, j, :])
                                nc.vector.tensor_mul(ex[:, 512:1024], ex[:, 512:1024], masks_sb[:, j, :])
                            exs[c] = ex
                        if c >= 1:
                            ex = exs[c - 1]
                            nc.tensor.matmul(
                                avA[:], lhsT=V65[:, c - 1, 2 * a, :], rhs=ex[:, 0:512],
                                start=(c == 1), stop=(c == ext),
                            )
                            nc.tensor.matmul(
                                avB[:], lhsT=V65[:, c - 1, 2 * a + 1, :], rhs=ex[:, 512:1024],
                                start=(c == 1), stop=(c == ext),
                            )
                    for hh, av in ((2 * a, avA), (2 * a + 1, avB)):
                        # copy PSUM out quickly, then build 1/denominator with
                        # the free dim spread across partitions (reciprocal is
                        # ~6.5 cyc per free-elem per lane, so [1,512] is slow)
                        avs = work.tile([D + 1, 512], F32, tag="avs")
                        nc.vector.tensor_copy(avs[:], av[:])
                        d4 = stats.tile([128, 4], F32, tag="d4")
                        nc.sync.dma_start(
                            d4[:], avs[D:D + 1, :].rearrange("o (p f) -> o p f", p=128)
                        )
                        r4 = stats.tile([128, 4], F32, tag="r4")
                        nc.vector.reciprocal(r4[:], d4[:])
                        rrow = stats.tile([1, 512], F32, tag="rrow")
                        nc.sync.dma_start(
                            rrow.rearrange("o (p f) -> o p f", p=128), r4[:]
                        )
                        den_b = work.tile([64, 512], F32, tag="denb")
                        nc.gpsimd.partition_broadcast(den_b[:], rrow[0:1, :], channels=64)
                        if hh % 2 == 0:
                            nc.vector.tensor_tensor(
                                attnT[0:64, a, :], avs[0:D, :], den_b[:], op=ALU.mult,
                            )
                        else:
                            tmp = work.tile([64, 512], BF16, tag="atmp")
                            nc.vector.tensor_tensor(tmp[:], avs[0:D, :], den_b[:], op=ALU.mult)
                            nc.sync.dma_start(attnT[64:128, a, :], tmp[:])

            def wo1_rs_block(qt):
                attnT = attnT_tiles[qt]
                for lt in range(4):
                    tt = 4 * qt + lt
                    ps = psC.tile([128, 512], F32, tag="psC")
                    for ko in range(2):
                        nc.tensor.matmul(
                            ps[:],
                            lhsT=attnT[:, ko, lt * 128:(lt + 1) * 128],
                            rhs=wo1_sb[:, ko, :],
                            start=(ko == 0), stop=(ko == 1),
                        )
                    o1t = work.tile([128, E], BF16, tag="wbf")
                    nc.vector.tensor_copy(o1t[:], ps[:])
                    nc.sync.dma_start(o1_dram[tt * 128:(tt + 1) * 128, :], o1t[:])
                nc.gpsimd.collective_compute(
                    "ReduceScatter", ALU.add,
                    replica_groups=[[0, 1], [2, 3], [4, 5], [6, 7]],
                    ins=[o1_dram[qt * 512:(qt + 1) * 512, :].opt()],
                    outs=[o1r_dram[qt * 256:(qt + 1) * 256, :].opt()],
                )
                o1rT = acts.tile([128, 4, 256], BF16, tag=f"o1rT{qt}")
                o1rT_tiles[qt] = o1rT
                nc.sync.dma_start_transpose(
                    o1rT[:], o1r_dram[qt * 256:(qt + 1) * 256, :],
                )

            for qt in range(QTS):
                attention_block(qt)
                if qt == 0:
                    # late-phase weights: load now so the DMAs don't compete
                    # with the x/LN1 stream at kernel start
                    wo2_sb = load_const([128, 4, E], wo2, "wo2_sb")
                    w1_sb = load_const([128, 4, 4 * E], w1, "w1_sb")
                    w2_sb = load_const([128, 16, E], w2, "w2_sb")
                    xq_sb = acts.tile([128, NTQ, E], F32)
                    nc.scalar.dma_start(xq_sb[:], xq[:])
                if qt >= 1:
                    wo1_rs_block(qt - 1)   # one block behind: keeps PE stream unblocked
            wo1_rs_block(QTS - 1)

            # ---- wo2 + residual + LN2 (all blocks), then MLP -------------
            # h2T/m1T are per-half tiles so first-half MLP matmuls never wait
            # on second-half LayerNorm chains
            h2_dram = dram.tile([SQ, E], BF16)
            h2T_tiles = [
                acts.tile([128, 4, 512], BF16, tag=f"h2T{h}", name=f"h2T{h}")
                for h in range(2)
            ]
            m1T_tiles = [
                acts.tile([128, 16, 512], BF16, tag=f"m1T{h}", name=f"m1T{h}")
                for h in range(2)
            ]
            for tt in range(NTQ):
                ps = psC.tile([128, 512], F32, tag="psC")
                o1rT = o1rT_tiles[tt // 2]
                for ko in range(4):
                    nc.tensor.matmul(
                        ps[:],
                        lhsT=o1rT[:, ko, (tt % 2) * 128:(tt % 2 + 1) * 128],
                        rhs=wo2_sb[:, ko, :],
                        start=(ko == 0), stop=(ko == 3),
                    )
                h2r = work.tile([128, E], F32, tag="wf32")
                nc.vector.tensor_add(h2r[:], ps[:], xq_sb[:, tt, :])
                h2t = work.tile([128, E], BF16, tag="wbf")
                layernorm_tile(h2r[:], h2t)
                nc.sync.dma_start(h2_dram[tt * 128:(tt + 1) * 128, :], h2t[:])
                if tt % 4 == 3:
                    half = tt // 4
                    hsl = slice(half * 512, (half + 1) * 512)
                    nc.sync.dma_start_transpose(h2T_tiles[half][:], h2_dram[hsl, :])
            for half in range(2):
                h2T = h2T_tiles[half]
                m1T = m1T_tiles[half]
                for mi in range(16):
                    ps = psC.tile([128, 512], F32, tag="psC")
                    for ko in range(4):
                        nc.tensor.matmul(
                            ps[:],
                            lhsT=w1_sb[:, ko, mi * 128:(mi + 1) * 128],
                            rhs=h2T[:, ko, :],
                            start=(ko == 0), stop=(ko == 3),
                        )
                    nc.scalar.activation(m1T[:, mi, :], ps[:], AF.Gelu_apprx_tanh)
                for lt in range(4):
                    tt = 4 * half + lt
                    ps = psC.tile([128, 512], F32, tag="psC")
                    for ko in range(16):
                        nc.tensor.matmul(
                            ps[:],
                            lhsT=m1T[:, ko, lt * 128:(lt + 1) * 128],
                            rhs=w2_sb[:, ko, :],
                            start=(ko == 0), stop=(ko == 15),
                        )
                    ot = work.tile([128, E], F32, tag="wf32")
                    nc.vector.tensor_add(ot[:], ps[:], xq_sb[:, tt, :])
                    nc.sync.dma_start(out[tt * 128:(tt + 1) * 128, :], ot[:])

    nc.finalize()
    return nc


@functools.lru_cache(maxsize=1)
def _get_graph():
    return _build_graph()


def _bf16_kpm(a, p=128):
    """[K, M] fp32 -> contiguous [p, K//p, M] bf16 (SBUF (k p) layout)."""
    k, m = a.shape
    return np.ascontiguousarray(
        a.reshape(k // p, p, m).transpose(1, 0, 2)
    ).astype(ml_dtypes.bfloat16)


def _own_rows(rank):
    """Global row indices owned by a core after the per-block reduce-scatters."""
    return np.concatenate(
        [np.arange(512 * qt + 256 * rank, 512 * qt + 256 * rank + 256) for qt in range(QTS)]
    )


def _make_in_maps(x, wq, wk, wv, wo1, wo2, w1, w2, ln1_scale, ln2_scale):
    x = np.asarray(x, dtype=np.float32)
    wq = np.asarray(wq, dtype=np.float32).reshape(E, H * D)
    wk = np.asarray(wk, dtype=np.float32).reshape(E, H * D)
    wv = np.asarray(wv, dtype=np.float32).reshape(E, H * D)
    wo1 = np.asarray(wo1, dtype=np.float32).reshape(H * D, E)
    wo2 = np.asarray(wo2, dtype=np.float32)
    w1 = np.asarray(w1, dtype=np.float32)
    w2 = np.asarray(w2, dtype=np.float32)
    s1 = np.asarray(ln1_scale, dtype=np.float32)[:, None]
    s2 = np.asarray(ln2_scale, dtype=np.float32)[:, None]

    wq_s, wk_s, wv_s = s1 * wq, s1 * wk, s1 * wv
    w1_s = s2 * w1

    # causal mask patterns for diagonal 128-chunks within a 512 q-tile:
    # mask_j[p, f] = 1.0 iff (128j + p) <= f;  stored [p, j, f]
    iota_p = np.arange(128)[:, None]
    iota_f = np.arange(512)[None, :]
    mask_np = np.ascontiguousarray(np.stack(
        [(128 * j + iota_p <= iota_f).astype(np.float32) for j in range(4)]
    ).transpose(1, 0, 2)).astype(ml_dtypes.bfloat16)

    in_maps = []
    for c in range(8):
        b, g = c // 2, c % 2
        hd = slice(g * HG * D, (g + 1) * HG * D)
        rows = _own_rows(c % 2)
        xq_arr = np.ascontiguousarray(
            x[b][rows].reshape(NTQ, 128, E).transpose(1, 0, 2)
        )
        in_maps.append({
            "xf": np.ascontiguousarray(x[b]),
            "xq": xq_arr,
            "wq": _bf16_kpm(wq_s[:, hd]),
            "wk": _bf16_kpm(wk_s[:, hd]),
            "wv": _bf16_kpm(wv_s[:, hd]),
            "wo1": _bf16_kpm(wo1[hd, :]),
            "wo2": _bf16_kpm(wo2),
            "w1": _bf16_kpm(w1_s),
            "w2": _bf16_kpm(w2),
            "masks": mask_np,
        })
    return in_maps


def run(trace=False, **inputs):
    nc = _get_graph()
    in_maps = _make_in_maps(**inputs)
    res = run_bass_kernel_spmd(nc, in_maps, core_ids=list(range(8)), trace=trace)
    y = np.empty((B, S, E), dtype=np.float32)
    for c in range(8):
        b = c // 2
        y[b][_own_rows(c % 2)] = res.results[c]["out"]
    return y, res


def kernel(**inputs):
    y, _ = run(trace=False, **inputs)
    return y
